# revision 22
# baseline (speedup 1.0000x reference)
"""Trainium2 Bass kernel for 16-head causal attention prefill (B=2, T=2048, D=2048).

Sharding: 8 cores = 2 batches x 4 head-groups; core c handles batch c//4,
heads 4*(c%4) .. 4*(c%4)+3 (tensor-parallel over heads within a batch).
Each core computes its heads' QKV projection, causal attention, and the
partial output projection over its 512 feature columns; the host sums the
4 partial projections per batch (the "all-reduce" of the TP scheme) and
adds the projection bias.

All matmuls run in bf16 (PE streams 2 cols/cycle; fp32 PSUM accumulation).
Causal masking is multiplicative post-exp (0/1 bf16 mask), softmax
denominators come from ones-vector matmuls, and the attention inner loop is
software-pipelined (QK of chunk c+1 issued before PV of chunk c) so the
in-order PE never stalls on the ACT exp. k/v/y ship back as bf16; the host
assembles fp32 outputs. Returns (out, k, v) matching the reference module.
"""

import math

import numpy as np
import ml_dtypes

import concourse.bass as bass  # noqa: F401
import concourse.mybir as mybir
import concourse.tile as tile
from concourse import bacc
from concourse.bass_utils import run_bass_kernel_spmd

F32 = mybir.dt.float32
BF16 = mybir.dt.bfloat16
EXP = mybir.ActivationFunctionType.Exp
BF16NP = ml_dtypes.bfloat16

B, T, D = 2, 2048, 2048
H = 16
DH = 128  # head dim
HPC = 4  # heads per core
NCORES = 8
KT = D // 128  # 16 contraction tiles
TSB = T // 512  # 4 token superblocks
TB = T // 128  # 16 token blocks

_CACHE = {}


def _build():
    nc = bacc.Bacc("TRN2", target_bir_lowering=False, debug=False, num_devices=NCORES)

    xt_ext = nc.dram_tensor("xt", [KT, 128, T], BF16, kind="ExternalInput")
    wqk_ext = nc.dram_tensor("wqk", [8, KT, 128, 128], BF16, kind="ExternalInput")
    wv_ext = nc.dram_tensor("wv", [KT, 128, 512], BF16, kind="ExternalInput")
    bqk_ext = nc.dram_tensor("bqk", [128, 8], F32, kind="ExternalInput")
    bvb_ext = nc.dram_tensor("bvb", [128, 512], F32, kind="ExternalInput")
    wp_ext = nc.dram_tensor("wp", [HPC, 128, D], BF16, kind="ExternalInput")
    mask_ext = nc.dram_tensor("mask", [128, 896], BF16, kind="ExternalInput")
    ones_ext = nc.dram_tensor("ones", [128, 1], F32, kind="ExternalInput")

    kt_out_ext = nc.dram_tensor("kt_out", [HPC, 128, T], BF16, kind="ExternalOutput")
    v_out_ext = nc.dram_tensor("v_out", [TB, 128, 512], BF16, kind="ExternalOutput")
    y_out_ext = nc.dram_tensor("y_out", [TB, 128, T], BF16, kind="ExternalOutput")

    F32R = mybir.dt.float32r

    pe_last = [None]

    def pe(inst):
        # total order over PE instructions: explicit ldweights stay glued to
        # their non-self-loading matmuls (nothing may clobber the PE weight
        # registers in between)
        if pe_last[0] is not None:
            tile.add_dep_helper(inst.ins, pe_last[0].ins, False, reason="pe-order")
        pe_last[0] = inst
        return inst

    with tile.TileContext(nc) as tc:
        with (
            tc.tile_pool(name="const", bufs=1) as constp,
            tc.tile_pool(name="dram", bufs=1, space="DRAM") as dramp,
            tc.tile_pool(name="qk2", bufs=2) as qk2p,
            tc.tile_pool(name="v2", bufs=2) as v2p,
            tc.tile_pool(name="wpp", bufs=1) as wpp,
            tc.tile_pool(name="w1", bufs=2) as w1p,
            tc.tile_pool(name="st1", bufs=4) as st1p,
            tc.tile_pool(name="outu", bufs=1) as outup,
            tc.tile_pool(name="pt", bufs=4) as ptp,
            tc.tile_pool(name="sum2", bufs=4) as sum2p,
            tc.tile_pool(name="dn", bufs=2) as dnp,
            tc.tile_pool(name="st3", bufs=4) as st3p,
        ):
            mask_t = constp.tile([128, 896], BF16)
            nc.sync.dma_start(mask_t[:], mask_ext[:])
            ones_t = constp.tile([128, 1], F32R)
            nc.sync.dma_start(ones_t[:], ones_ext[:].bitcast(F32R))
            bqk_t = constp.tile([128, 8], F32)
            nc.sync.dma_start(bqk_t[:], bqk_ext[:])
            bvb_t = constp.tile([128, 512], F32)
            nc.sync.dma_start(bvb_t[:], bvb_ext[:])

            qk_spill = dramp.tile([8, 128, T], BF16)
            v_spill = dramp.tile([HPC, TB, 128, 128], BF16)  # head-major
            # outU[d, 4*h + tsb, t]: normalized attention output (bf16)
            outU = outup.tile([128, 16, 512], BF16)

            # ---------------- Phase 1a: V projection ----------------
            # v in natural [t, f] layout (f = 4 heads x 128); wv/xt loads
            # interleaved so the first matmul chain starts after ~1MB of DMA.
            with tc.tile_pool(name="xt", bufs=1) as xtp:
                xt_t = xtp.tile([128, KT, T], BF16)
                with (
                    tc.tile_pool(name="wv", bufs=1) as wvp,
                    tc.tile_pool(name="psv", bufs=4, space="PSUM") as psvp,
                ):
                    wv_t = wvp.tile([128, KT, 512], BF16)
                    for kt in range(KT):
                        nc.sync.dma_start(wv_t[:, kt, :], wv_ext[kt])
                        nc.sync.dma_start(xt_t[:, kt, :], xt_ext[kt])
                    for tb in range(TB):
                        psv = psvp.tile([128, 512], F32, tag="psv")
                        for kt in range(KT):
                            xsl = xt_t[:, kt, 128 * tb : 128 * (tb + 1)]
                            pe(nc.tensor.ldweights(xsl))
                            mi = nc.tensor.matmul(
                                psv[:],
                                xsl,
                                wv_t[:, kt, :],
                                start=(kt == 0),
                                stop=(kt == KT - 1),
                            )
                            mi.ins.ldweights = False
                            pe(mi)
                        stgv = st1p.tile([128, 512], BF16, tag="stg")
                        nc.vector.tensor_add(stgv[:], psv[:], bvb_t[:])
                        for j in range(HPC):
                            nc.sync.dma_start(
                                v_spill[j, tb], stgv[:, 128 * j : 128 * (j + 1)]
                            )
                        nc.sync.dma_start(v_out_ext[tb], stgv[:])

                # ------ Phase 1b + 2 + 3 interleaved ------
                with (
                    tc.tile_pool(name="ps1", bufs=1, space="PSUM") as ps1p,
                    tc.tile_pool(name="ps_s", bufs=2, space="PSUM") as pssp,
                    tc.tile_pool(name="ps_o", bufs=1, space="PSUM") as psop,
                    tc.tile_pool(name="ps_d", bufs=1, space="PSUM") as psdp,
                ):
                    wp_t = wpp.tile([128, HPC, D], BF16)

                    def emit_qk(fb):
                        # q (fb 0-3, scaled by 1/sqrt(dh)) / k (fb 4-7) in
                        # qT/kT[f, t] layout; two tsb accumulations per pass.
                        w_t = w1p.tile([128, KT, 128], BF16, tag="w1", name=f"w_{fb}")
                        for kt in range(KT):
                            nc.sync.dma_start(w_t[:, kt, :], wqk_ext[fb, kt])
                        for half in range(2):
                            ps_a = ps1p.tile(
                                [128, 512], F32, tag="ps1_a", name=f"psa_{fb}_{half}"
                            )
                            ps_b = ps1p.tile(
                                [128, 512], F32, tag="ps1_b", name=f"psb_{fb}_{half}"
                            )
                            for kt in range(KT):
                                pe(nc.tensor.ldweights(w_t[:, kt, :]))
                                for tsb, ps in ((2 * half, ps_a), (2 * half + 1, ps_b)):
                                    mi = nc.tensor.matmul(
                                        ps[:],
                                        w_t[:, kt, :],
                                        xt_t[:, kt, 512 * tsb : 512 * (tsb + 1)],
                                        start=(kt == 0),
                                        stop=(kt == KT - 1),
                                    )
                                    mi.ins.ldweights = False
                                    pe(mi)
                            for tsb, ps in ((2 * half, ps_a), (2 * half + 1, ps_b)):
                                stg = st1p.tile([128, 512], BF16, tag="stg")
                                nc.vector.tensor_scalar_add(
                                    stg[:], ps[:], bqk_t[:, fb : fb + 1]
                                )
                                sl = slice(512 * tsb, 512 * (tsb + 1))
                                nc.sync.dma_start(qk_spill[fb, :, sl], stg[:])
                                if fb >= 4:
                                    nc.sync.dma_start(kt_out_ext[fb - 4, :, sl], stg[:])

                    def emit_attention(h, with_proj):
                        q_t = qk2p.tile([128, T], BF16, tag="q", name=f"q_{h}")
                        nc.sync.dma_start(q_t[:], qk_spill[h])
                        k_t = qk2p.tile([128, T], BF16, tag="k", name=f"k_{h}")
                        nc.sync.dma_start(k_t[:], qk_spill[4 + h])
                        v_t = v2p.tile([128, TB, 128], BF16, tag="v", name=f"v_{h}")
                        for tb in range(TB):
                            nc.sync.dma_start(v_t[:, tb, :], v_spill[h, tb])

                        for tsb in range(TSB):
                            nt = 4 * (tsb + 1)  # causal tk blocks
                            nch = nt // 2  # chunks of 2 tk blocks
                            ps_o = psop.tile(
                                [128, 512], F32, tag="ps_o", name=f"pso_{h}_{tsb}"
                            )
                            ps_d = psdp.tile(
                                [1, 512], F32, tag="ps_d", name=f"psd_{h}_{tsb}"
                            )
                            qsl = q_t[:, 512 * tsb : 512 * (tsb + 1)]

                            # software pipeline: QK+exp of chunk c before
                            # PV/denominator of chunk c-1
                            pending = None
                            for c in range(nch + 1):
                                if c < nch:
                                    ps_c = pssp.tile(
                                        [128, 1024], F32, tag="ps_s",
                                        name=f"pss_{h}_{tsb}_{c}",
                                    )
                                    for j in range(2):
                                        tkb = 2 * c + j
                                        ksl = k_t[:, 128 * tkb : 128 * (tkb + 1)]
                                        pe(nc.tensor.ldweights(ksl))
                                        mi = nc.tensor.matmul(
                                            ps_c[:, 512 * j : 512 * (j + 1)],
                                            ksl,
                                            qsl,
                                            start=True,
                                            stop=True,
                                        )
                                        mi.ins.ldweights = False
                                        pe(mi)
                                    pt_c = ptp.tile([128, 1024], BF16, tag="pt")
                                    nc.scalar.activation(pt_c[:], ps_c[:], EXP)
                                    for j in range(2):
                                        tkb = 2 * c + j
                                        if tkb >= 4 * tsb:  # diagonal: 0/1 mask
                                            i = tkb - 4 * tsb
                                            nc.vector.tensor_mul(
                                                pt_c[:, 512 * j : 512 * (j + 1)],
                                                pt_c[:, 512 * j : 512 * (j + 1)],
                                                mask_t[
                                                    :, 384 - 128 * i : 896 - 128 * i
                                                ],
                                            )
                                    # chunk presum: one f32r denominator matmul
                                    # per chunk instead of one per block
                                    sum2 = sum2p.tile([128, 512], F32R, tag="sum2")
                                    nc.vector.tensor_add(
                                        sum2[:], pt_c[:, :512], pt_c[:, 512:]
                                    )
                                if pending is not None:
                                    pp, psum2, c0 = pending
                                    for j in range(2):
                                        tkb = 2 * c0 + j
                                        pe(nc.tensor.ldweights(v_t[:, tkb, :]))
                                        mi = nc.tensor.matmul(
                                            ps_o[:],
                                            v_t[:, tkb, :],
                                            pp[:, 512 * j : 512 * (j + 1)],
                                            start=(tkb == 0),
                                            stop=(tkb == nt - 1),
                                        )
                                        mi.ins.ldweights = False
                                        pe(mi)
                                    pe(
                                        nc.tensor.matmul(
                                            ps_d[:],
                                            ones_t[:],
                                            psum2[:],
                                            start=(c0 == 0),
                                            stop=(c0 == nch - 1),
                                        )
                                    )
                                if c < nch:
                                    pending = (pt_c, sum2, c)

                            # denominator -> broadcast -> reciprocal -> normalize
                            d_sb = dnp.tile([1, 512], F32, tag="dsb")
                            nc.vector.tensor_copy(d_sb[:], ps_d[:])
                            D_t = dnp.tile([128, 512], F32, tag="Dt")
                            nc.gpsimd.partition_broadcast(D_t[:], d_sb[:])
                            r_t = dnp.tile([128, 512], F32, tag="rt")
                            nc.vector.reciprocal_approx_fast(r_t[:], D_t[:])
                            nc.vector.tensor_mul(
                                outU[:, 4 * h + tsb, :], ps_o[:], r_t[:]
                            )
                            if with_proj:
                                emit_proj(tsb)

                    def emit_proj(tsb):
                        for db in range(TB):
                            ps_y = pssp.tile(
                                [128, 512], F32, tag="ps_s", name=f"psy_{tsb}_{db}"
                            )
                            for kt in range(HPC):
                                wsl = wp_t[:, kt, 128 * db : 128 * (db + 1)]
                                pe(nc.tensor.ldweights(wsl))
                                mi = nc.tensor.matmul(
                                    ps_y[:],
                                    wsl,
                                    outU[:, 4 * kt + tsb, :],
                                    start=(kt == 0),
                                    stop=(kt == HPC - 1),
                                )
                                mi.ins.ldweights = False
                                pe(mi)
                            stg = st3p.tile([128, 512], BF16, tag="st3")
                            if db % 2 == 0:
                                nc.vector.tensor_copy(stg[:], ps_y[:])
                            else:
                                nc.scalar.activation(
                                    stg[:],
                                    ps_y[:],
                                    mybir.ActivationFunctionType.Copy,
                                )
                            nc.sync.dma_start(
                                y_out_ext[db, :, 512 * tsb : 512 * (tsb + 1)],
                                stg[:],
                            )

                    # interleave: q/k projection pairs with attention heads so
                    # attention ACT/DVE work overlaps projection matmuls and
                    # DMA stalls are absorbed.
                    emit_qk(0)
                    emit_qk(4)
                    emit_qk(1)
                    emit_qk(5)
                    for kt in range(HPC):
                        nc.sync.dma_start(wp_t[:, kt, :], wp_ext[kt])
                    emit_attention(0, False)
                    emit_qk(2)
                    emit_qk(6)
                    emit_attention(1, False)
                    emit_qk(3)
                    emit_qk(7)
                    emit_attention(2, False)
                    emit_attention(3, True)

    nc.compile()
    return nc


def _prep_in_maps(inputs, w_qkv, b_qkv, w_proj):
    scale = 1.0 / math.sqrt(DH)
    mask = np.zeros((128, 896), np.float32)
    p_idx = np.arange(128)[:, None]
    c_idx = np.arange(896)[None, :]
    mask[p_idx <= c_idx - 384] = 1.0
    ones = np.ones((128, 1), np.float32)

    wqkT = np.ascontiguousarray(w_qkv.T)  # [D, 3D]
    wpT = np.ascontiguousarray(w_proj.T)  # [D, D]

    in_maps = []
    for c in range(NCORES):
        b = c // 4
        hg = c % 4
        heads = [4 * hg + j for j in range(HPC)]

        xt = np.ascontiguousarray(inputs[b].T).reshape(KT, 128, T)

        wqk = np.empty((8, KT, 128, 128), np.float32)
        bqk = np.empty((128, 8), np.float32)
        wv = np.empty((KT, 128, HPC, 128), np.float32)
        bvb = np.empty((128, HPC, 128), np.float32)
        for j, h in enumerate(heads):
            r0 = h * 3 * DH
            wqk[j] = (wqkT[:, r0 : r0 + DH] * scale).reshape(KT, 128, 128)
            wqk[4 + j] = wqkT[:, r0 + DH : r0 + 2 * DH].reshape(KT, 128, 128)
            bqk[:, j] = b_qkv[r0 : r0 + DH] * scale
            bqk[:, 4 + j] = b_qkv[r0 + DH : r0 + 2 * DH]
            wv[:, :, j, :] = wqkT[:, r0 + 2 * DH : r0 + 3 * DH].reshape(KT, 128, 128)
            bvb[:, j, :] = b_qkv[r0 + 2 * DH : r0 + 3 * DH][None, :]
        wv = wv.reshape(KT, 128, 512)
        bvb = bvb.reshape(128, 512)

        d0 = 4 * hg * DH
        wp = np.ascontiguousarray(wpT[d0 : d0 + 512, :]).reshape(HPC, 128, D)

        in_maps.append(
            {
                "xt": xt.astype(BF16NP),
                "wqk": wqk.astype(BF16NP),
                "wv": wv.astype(BF16NP),
                "bqk": np.ascontiguousarray(bqk),
                "bvb": np.ascontiguousarray(bvb),
                "wp": wp.astype(BF16NP),
                "mask": mask.astype(BF16NP),
                "ones": ones,
            }
        )
    return in_maps


def kernel_run(inputs, w_qkv, b_qkv, w_proj, b_proj, trace=False):
    """Run the kernel; returns ((out, k, v), exec_time_ns)."""
    inputs = np.asarray(inputs, np.float32)
    w_qkv = np.asarray(w_qkv, np.float32)
    b_qkv = np.asarray(b_qkv, np.float32)
    w_proj = np.asarray(w_proj, np.float32)
    b_proj = np.asarray(b_proj, np.float32)

    if "nc" not in _CACHE:
        _CACHE["nc"] = _build()
    nc = _CACHE["nc"]

    in_maps = _prep_in_maps(inputs, w_qkv, b_qkv, w_proj)
    res = run_bass_kernel_spmd(
        nc, in_maps, core_ids=list(range(NCORES)), trace=trace
    )
    outs = res.results

    k = np.empty((B, H, T, DH), np.float32)
    v = np.empty((B, H, T, DH), np.float32)
    y = np.zeros((B, T, D), np.float32)
    for c in range(NCORES):
        b = c // 4
        hg = c % 4
        kt_o = np.asarray(outs[c]["kt_out"], dtype=np.float32)  # [4, 128, T]
        v_o = np.asarray(outs[c]["v_out"], dtype=np.float32).reshape(T, 512)
        for j in range(HPC):
            k[b, 4 * hg + j] = kt_o[j].T
            v[b, 4 * hg + j] = v_o[:, 128 * j : 128 * (j + 1)]
        y[b] += np.asarray(outs[c]["y_out"], dtype=np.float32).reshape(D, T).T
    y += b_proj[None, None, :]
    return (y, k, v), res.exec_time_ns


def kernel(inputs, w_qkv, b_qkv, w_proj, b_proj):
    out, _ = kernel_run(inputs, w_qkv, b_qkv, w_proj, b_proj, trace=False)
    return out


# revision 23
# speedup vs baseline: 1.0531x; 1.0531x over previous
"""Trainium2 Bass kernel for 16-head causal attention prefill (B=2, T=2048, D=2048).

Sharding: 8 cores = 2 batches x 4 head-groups; core c handles batch c//4,
heads 4*(c%4) .. 4*(c%4)+3 (tensor-parallel over heads within a batch).
Each core computes its heads' QKV projection, causal attention, and the
partial output projection over its 512 feature columns; the host sums the
4 partial projections per batch (the "all-reduce" of the TP scheme) and
adds the projection bias.

All matmuls run in bf16 (PE streams 2 cols/cycle; fp32 PSUM accumulation).
Causal masking is multiplicative post-exp (0/1 bf16 mask), softmax
denominators come from ones-vector matmuls, and the attention inner loop is
software-pipelined (QK of chunk c+1 issued before PV of chunk c) so the
in-order PE never stalls on the ACT exp. k/v/y ship back as bf16; the host
assembles fp32 outputs. Returns (out, k, v) matching the reference module.
"""

import math

import numpy as np
import ml_dtypes

import concourse.bass as bass  # noqa: F401
import concourse.mybir as mybir
import concourse.tile as tile
from concourse import bacc
from concourse.bass_utils import run_bass_kernel_spmd

F32 = mybir.dt.float32
BF16 = mybir.dt.bfloat16
EXP = mybir.ActivationFunctionType.Exp
BF16NP = ml_dtypes.bfloat16

B, T, D = 2, 2048, 2048
H = 16
DH = 128  # head dim
HPC = 4  # heads per core
NCORES = 8
KT = D // 128  # 16 contraction tiles
TSB = T // 512  # 4 token superblocks
TB = T // 128  # 16 token blocks

_CACHE = {}


def _build():
    nc = bacc.Bacc("TRN2", target_bir_lowering=False, debug=False, num_devices=NCORES)

    xt_ext = nc.dram_tensor("xt", [KT, 128, T], BF16, kind="ExternalInput")
    wqk_ext = nc.dram_tensor("wqk", [8, KT, 128, 128], BF16, kind="ExternalInput")
    wv_ext = nc.dram_tensor("wv", [KT, 128, 512], BF16, kind="ExternalInput")
    bqk_ext = nc.dram_tensor("bqk", [128, 8], F32, kind="ExternalInput")
    bvb_ext = nc.dram_tensor("bvb", [128, 512], F32, kind="ExternalInput")
    wp_ext = nc.dram_tensor("wp", [HPC, 128, D], BF16, kind="ExternalInput")
    mask_ext = nc.dram_tensor("mask", [128, 896], BF16, kind="ExternalInput")
    ones_ext = nc.dram_tensor("ones", [128, 1], F32, kind="ExternalInput")

    kt_out_ext = nc.dram_tensor("kt_out", [HPC, 128, T], BF16, kind="ExternalOutput")
    v_out_ext = nc.dram_tensor("v_out", [TB, 128, 512], BF16, kind="ExternalOutput")
    y_out_ext = nc.dram_tensor("y_out", [TB, 128, T], BF16, kind="ExternalOutput")

    F32R = mybir.dt.float32r

    with tile.TileContext(nc) as tc:
        with (
            tc.tile_pool(name="const", bufs=1) as constp,
            tc.tile_pool(name="dram", bufs=1, space="DRAM") as dramp,
            tc.tile_pool(name="qk2", bufs=2) as qk2p,
            tc.tile_pool(name="v2", bufs=2) as v2p,
            tc.tile_pool(name="wpp", bufs=1) as wpp,
            tc.tile_pool(name="w1", bufs=2) as w1p,
            tc.tile_pool(name="st1", bufs=4) as st1p,
            tc.tile_pool(name="outu", bufs=1) as outup,
            tc.tile_pool(name="pt", bufs=6) as ptp,
            tc.tile_pool(name="sum2", bufs=4) as sum2p,
            tc.tile_pool(name="dn", bufs=2) as dnp,
            tc.tile_pool(name="st3", bufs=4) as st3p,
        ):
            mask_t = constp.tile([128, 896], BF16)
            nc.sync.dma_start(mask_t[:], mask_ext[:])
            ones_t = constp.tile([128, 1], F32R)
            nc.sync.dma_start(ones_t[:], ones_ext[:].bitcast(F32R))
            bqk_t = constp.tile([128, 8], F32)
            nc.sync.dma_start(bqk_t[:], bqk_ext[:])
            bvb_t = constp.tile([128, 512], F32)
            nc.sync.dma_start(bvb_t[:], bvb_ext[:])

            qk_spill = dramp.tile([8, 128, T], BF16)
            v_spill = dramp.tile([HPC, TB, 128, 128], BF16)  # head-major
            # outU[d, 4*h + tsb, t]: normalized attention output (bf16)
            outU = outup.tile([128, 16, 512], BF16)

            # ---------------- Phase 1a: V projection ----------------
            # v in natural [t, f] layout (f = 4 heads x 128); wv/xt loads
            # interleaved so the first matmul chain starts after ~1MB of DMA.
            with tc.tile_pool(name="xt", bufs=1) as xtp:
                xt_t = xtp.tile([128, KT, T], BF16)
                with (
                    tc.tile_pool(name="wv", bufs=1) as wvp,
                    tc.tile_pool(name="psv", bufs=4, space="PSUM") as psvp,
                ):
                    wv_t = wvp.tile([128, KT, 512], BF16)
                    for kt in range(KT):
                        nc.sync.dma_start(wv_t[:, kt, :], wv_ext[kt])
                        nc.sync.dma_start(xt_t[:, kt, :], xt_ext[kt])
                    for tb in range(TB):
                        psv = psvp.tile([128, 512], F32, tag="psv")
                        for kt in range(KT):
                            nc.tensor.matmul(
                                psv[:],
                                xt_t[:, kt, 128 * tb : 128 * (tb + 1)],
                                wv_t[:, kt, :],
                                start=(kt == 0),
                                stop=(kt == KT - 1),
                            )
                        stgv = st1p.tile([128, 512], BF16, tag="stg")
                        nc.vector.tensor_add(stgv[:], psv[:], bvb_t[:])
                        for j in range(HPC):
                            nc.sync.dma_start(
                                v_spill[j, tb], stgv[:, 128 * j : 128 * (j + 1)]
                            )
                        nc.sync.dma_start(v_out_ext[tb], stgv[:])

                # ------ Phase 1b + 2 + 3 interleaved ------
                with (
                    tc.tile_pool(name="ps1", bufs=1, space="PSUM") as ps1p,
                    tc.tile_pool(name="ps_s", bufs=4, space="PSUM") as pssp,
                    tc.tile_pool(name="ps_o", bufs=1, space="PSUM") as psop,
                    tc.tile_pool(name="ps_d", bufs=1, space="PSUM") as psdp,
                ):
                    wp_t = wpp.tile([128, HPC, D], BF16)

                    def emit_qk(fb):
                        # q (fb 0-3, scaled by 1/sqrt(dh)) / k (fb 4-7) in
                        # qT/kT[f, t] layout; two tsb accumulations per pass.
                        w_t = w1p.tile([128, KT, 128], BF16, tag="w1", name=f"w_{fb}")
                        for kt in range(KT):
                            nc.sync.dma_start(w_t[:, kt, :], wqk_ext[fb, kt])
                        for half in range(2):
                            ps_a = ps1p.tile(
                                [128, 512], F32, tag="ps1_a", name=f"psa_{fb}_{half}"
                            )
                            ps_b = ps1p.tile(
                                [128, 512], F32, tag="ps1_b", name=f"psb_{fb}_{half}"
                            )
                            for kt in range(KT):
                                for tsb, ps in ((2 * half, ps_a), (2 * half + 1, ps_b)):
                                    nc.tensor.matmul(
                                        ps[:],
                                        w_t[:, kt, :],
                                        xt_t[:, kt, 512 * tsb : 512 * (tsb + 1)],
                                        start=(kt == 0),
                                        stop=(kt == KT - 1),
                                    )
                            for tsb, ps in ((2 * half, ps_a), (2 * half + 1, ps_b)):
                                stg = st1p.tile([128, 512], BF16, tag="stg")
                                nc.vector.tensor_scalar_add(
                                    stg[:], ps[:], bqk_t[:, fb : fb + 1]
                                )
                                sl = slice(512 * tsb, 512 * (tsb + 1))
                                nc.sync.dma_start(qk_spill[fb, :, sl], stg[:])
                                if fb >= 4:
                                    nc.sync.dma_start(kt_out_ext[fb - 4, :, sl], stg[:])

                    def emit_attention(h, with_proj):
                        q_t = qk2p.tile([128, T], BF16, tag="q", name=f"q_{h}")
                        nc.sync.dma_start(q_t[:], qk_spill[h])
                        k_t = qk2p.tile([128, T], BF16, tag="k", name=f"k_{h}")
                        nc.sync.dma_start(k_t[:], qk_spill[4 + h])
                        v_t = v2p.tile([128, TB, 128], BF16, tag="v", name=f"v_{h}")
                        for tb in range(TB):
                            nc.sync.dma_start(v_t[:, tb, :], v_spill[h, tb])

                        for tsb in range(TSB):
                            nt = 4 * (tsb + 1)  # causal tk blocks
                            nch = nt // 2  # chunks of 2 tk blocks
                            ps_o = psop.tile(
                                [128, 512], F32, tag="ps_o", name=f"pso_{h}_{tsb}"
                            )
                            ps_d = psdp.tile(
                                [1, 512], F32, tag="ps_d", name=f"psd_{h}_{tsb}"
                            )
                            qsl = q_t[:, 512 * tsb : 512 * (tsb + 1)]

                            # software pipeline: QK+exp of chunk c before
                            # PV/denominator of chunk c-1
                            pending = None
                            for c in range(nch + 1):
                                if c < nch:
                                    pts = []
                                    for j in range(2):
                                        tkb = 2 * c + j
                                        ps_s = pssp.tile(
                                            [128, 512], F32, tag="ps_s",
                                            name=f"pss_{h}_{tsb}_{c}_{j}",
                                        )
                                        nc.tensor.matmul(
                                            ps_s[:],
                                            k_t[:, 128 * tkb : 128 * (tkb + 1)],
                                            qsl,
                                            start=True,
                                            stop=True,
                                        )
                                        pt_j = ptp.tile(
                                            [128, 512], BF16, tag="pt",
                                            name=f"pt_{h}_{tsb}_{c}_{j}",
                                        )
                                        nc.scalar.activation(pt_j[:], ps_s[:], EXP)
                                        if tkb >= 4 * tsb:  # diagonal: 0/1 mask
                                            i = tkb - 4 * tsb
                                            nc.vector.tensor_mul(
                                                pt_j[:],
                                                pt_j[:],
                                                mask_t[
                                                    :, 384 - 128 * i : 896 - 128 * i
                                                ],
                                            )
                                        pts.append(pt_j)
                                    # chunk presum: one f32r denominator matmul
                                    # per chunk instead of one per block
                                    sum2 = sum2p.tile([128, 512], F32R, tag="sum2")
                                    nc.vector.tensor_add(
                                        sum2[:], pts[0][:], pts[1][:]
                                    )
                                if pending is not None:
                                    pp, psum2, c0 = pending
                                    for j in range(2):
                                        tkb = 2 * c0 + j
                                        nc.tensor.matmul(
                                            ps_o[:],
                                            v_t[:, tkb, :],
                                            pp[j][:],
                                            start=(tkb == 0),
                                            stop=(tkb == nt - 1),
                                        )
                                    nc.tensor.matmul(
                                        ps_d[:],
                                        ones_t[:],
                                        psum2[:],
                                        start=(c0 == 0),
                                        stop=(c0 == nch - 1),
                                    )
                                if c < nch:
                                    pending = (pts, sum2, c)

                            # denominator -> broadcast -> reciprocal -> normalize
                            d_sb = dnp.tile([1, 512], F32, tag="dsb")
                            nc.vector.tensor_copy(d_sb[:], ps_d[:])
                            D_t = dnp.tile([128, 512], F32, tag="Dt")
                            nc.gpsimd.partition_broadcast(D_t[:], d_sb[:])
                            r_t = dnp.tile([128, 512], F32, tag="rt")
                            nc.vector.reciprocal_approx_fast(r_t[:], D_t[:])
                            nc.vector.tensor_mul(
                                outU[:, 4 * h + tsb, :], ps_o[:], r_t[:]
                            )
                            if with_proj:
                                emit_proj(tsb)

                    def emit_proj(tsb):
                        for db in range(TB):
                            ps_y = pssp.tile(
                                [128, 512], F32, tag="ps_s", name=f"psy_{tsb}_{db}"
                            )
                            for kt in range(HPC):
                                nc.tensor.matmul(
                                    ps_y[:],
                                    wp_t[:, kt, 128 * db : 128 * (db + 1)],
                                    outU[:, 4 * kt + tsb, :],
                                    start=(kt == 0),
                                    stop=(kt == HPC - 1),
                                )
                            stg = st3p.tile([128, 512], BF16, tag="st3")
                            if db % 2 == 0:
                                nc.vector.tensor_copy(stg[:], ps_y[:])
                            else:
                                nc.scalar.activation(
                                    stg[:],
                                    ps_y[:],
                                    mybir.ActivationFunctionType.Copy,
                                )
                            nc.sync.dma_start(
                                y_out_ext[db, :, 512 * tsb : 512 * (tsb + 1)],
                                stg[:],
                            )

                    # interleave: q/k projection pairs with attention heads so
                    # attention ACT/DVE work overlaps projection matmuls and
                    # DMA stalls are absorbed.
                    emit_qk(0)
                    emit_qk(4)
                    emit_qk(1)
                    emit_qk(5)
                    for kt in range(HPC):
                        nc.sync.dma_start(wp_t[:, kt, :], wp_ext[kt])
                    emit_attention(0, False)
                    emit_qk(2)
                    emit_qk(6)
                    emit_attention(1, False)
                    emit_qk(3)
                    emit_qk(7)
                    emit_attention(2, False)
                    emit_attention(3, True)

    nc.compile()
    return nc


def _prep_in_maps(inputs, w_qkv, b_qkv, w_proj):
    scale = 1.0 / math.sqrt(DH)
    mask = np.zeros((128, 896), np.float32)
    p_idx = np.arange(128)[:, None]
    c_idx = np.arange(896)[None, :]
    mask[p_idx <= c_idx - 384] = 1.0
    ones = np.ones((128, 1), np.float32)

    wqkT = np.ascontiguousarray(w_qkv.T)  # [D, 3D]
    wpT = np.ascontiguousarray(w_proj.T)  # [D, D]

    in_maps = []
    for c in range(NCORES):
        b = c // 4
        hg = c % 4
        heads = [4 * hg + j for j in range(HPC)]

        xt = np.ascontiguousarray(inputs[b].T).reshape(KT, 128, T)

        wqk = np.empty((8, KT, 128, 128), np.float32)
        bqk = np.empty((128, 8), np.float32)
        wv = np.empty((KT, 128, HPC, 128), np.float32)
        bvb = np.empty((128, HPC, 128), np.float32)
        for j, h in enumerate(heads):
            r0 = h * 3 * DH
            wqk[j] = (wqkT[:, r0 : r0 + DH] * scale).reshape(KT, 128, 128)
            wqk[4 + j] = wqkT[:, r0 + DH : r0 + 2 * DH].reshape(KT, 128, 128)
            bqk[:, j] = b_qkv[r0 : r0 + DH] * scale
            bqk[:, 4 + j] = b_qkv[r0 + DH : r0 + 2 * DH]
            wv[:, :, j, :] = wqkT[:, r0 + 2 * DH : r0 + 3 * DH].reshape(KT, 128, 128)
            bvb[:, j, :] = b_qkv[r0 + 2 * DH : r0 + 3 * DH][None, :]
        wv = wv.reshape(KT, 128, 512)
        bvb = bvb.reshape(128, 512)

        d0 = 4 * hg * DH
        wp = np.ascontiguousarray(wpT[d0 : d0 + 512, :]).reshape(HPC, 128, D)

        in_maps.append(
            {
                "xt": xt.astype(BF16NP),
                "wqk": wqk.astype(BF16NP),
                "wv": wv.astype(BF16NP),
                "bqk": np.ascontiguousarray(bqk),
                "bvb": np.ascontiguousarray(bvb),
                "wp": wp.astype(BF16NP),
                "mask": mask.astype(BF16NP),
                "ones": ones,
            }
        )
    return in_maps


def kernel_run(inputs, w_qkv, b_qkv, w_proj, b_proj, trace=False):
    """Run the kernel; returns ((out, k, v), exec_time_ns)."""
    inputs = np.asarray(inputs, np.float32)
    w_qkv = np.asarray(w_qkv, np.float32)
    b_qkv = np.asarray(b_qkv, np.float32)
    w_proj = np.asarray(w_proj, np.float32)
    b_proj = np.asarray(b_proj, np.float32)

    if "nc" not in _CACHE:
        _CACHE["nc"] = _build()
    nc = _CACHE["nc"]

    in_maps = _prep_in_maps(inputs, w_qkv, b_qkv, w_proj)
    res = run_bass_kernel_spmd(
        nc, in_maps, core_ids=list(range(NCORES)), trace=trace
    )
    outs = res.results

    k = np.empty((B, H, T, DH), np.float32)
    v = np.empty((B, H, T, DH), np.float32)
    y = np.zeros((B, T, D), np.float32)
    for c in range(NCORES):
        b = c // 4
        hg = c % 4
        kt_o = np.asarray(outs[c]["kt_out"], dtype=np.float32)  # [4, 128, T]
        v_o = np.asarray(outs[c]["v_out"], dtype=np.float32).reshape(T, 512)
        for j in range(HPC):
            k[b, 4 * hg + j] = kt_o[j].T
            v[b, 4 * hg + j] = v_o[:, 128 * j : 128 * (j + 1)]
        y[b] += np.asarray(outs[c]["y_out"], dtype=np.float32).reshape(D, T).T
    y += b_proj[None, None, :]
    return (y, k, v), res.exec_time_ns


def kernel(inputs, w_qkv, b_qkv, w_proj, b_proj):
    out, _ = kernel_run(inputs, w_qkv, b_qkv, w_proj, b_proj, trace=False)
    return out


# revision 24
# speedup vs baseline: 1.2035x; 1.1428x over previous
"""Trainium2 Bass kernel for 16-head causal attention prefill (B=2, T=2048, D=2048).

Sharding: 8 cores = 2 batches x 4 head-groups; core c handles batch c//4,
heads 4*(c%4) .. 4*(c%4)+3 (tensor-parallel over heads within a batch).
Each core computes its heads' QKV projection, causal attention, and the
partial output projection over its 512 feature columns; the host sums the
4 partial projections per batch (the "all-reduce" of the TP scheme) and
adds the projection bias.

All matmuls run in bf16 (PE streams 2 cols/cycle; fp32 PSUM accumulation).
Causal masking is multiplicative post-exp (0/1 bf16 mask), softmax
denominators come from ones-vector matmuls, and the attention inner loop is
software-pipelined (QK of chunk c+1 issued before PV of chunk c) so the
in-order PE never stalls on the ACT exp. k/v/y ship back as bf16; the host
assembles fp32 outputs. Returns (out, k, v) matching the reference module.
"""

import math

import numpy as np
import ml_dtypes

import concourse.bass as bass  # noqa: F401
import concourse.mybir as mybir
import concourse.tile as tile
from concourse import bacc
from concourse.bass_utils import run_bass_kernel_spmd

F32 = mybir.dt.float32
BF16 = mybir.dt.bfloat16
EXP = mybir.ActivationFunctionType.Exp
BF16NP = ml_dtypes.bfloat16

B, T, D = 2, 2048, 2048
H = 16
DH = 128  # head dim
HPC = 4  # heads per core
NCORES = 8
KT = D // 128  # 16 contraction tiles
TSB = T // 512  # 4 token superblocks
TB = T // 128  # 16 token blocks

_CACHE = {}


def _build():
    nc = bacc.Bacc("TRN2", target_bir_lowering=False, debug=False, num_devices=NCORES)

    xt_ext = nc.dram_tensor("xt", [KT, 128, T], BF16, kind="ExternalInput")
    wqk_ext = nc.dram_tensor("wqk", [8, KT, 128, 128], BF16, kind="ExternalInput")
    wv_ext = nc.dram_tensor("wv", [KT, 128, 512], BF16, kind="ExternalInput")
    bqk_ext = nc.dram_tensor("bqk", [128, 8], F32, kind="ExternalInput")
    bvb_ext = nc.dram_tensor("bvb", [128, 512], F32, kind="ExternalInput")
    wp_ext = nc.dram_tensor("wp", [HPC, 128, D], BF16, kind="ExternalInput")
    mask_ext = nc.dram_tensor("mask", [128, 896], BF16, kind="ExternalInput")
    ones_ext = nc.dram_tensor("ones", [128, 1], F32, kind="ExternalInput")

    kt_out_ext = nc.dram_tensor("kt_out", [HPC, 128, T], BF16, kind="ExternalOutput")
    v_out_ext = nc.dram_tensor("v_out", [TB, 128, 512], BF16, kind="ExternalOutput")
    y_out_ext = nc.dram_tensor("y_out", [TB, 128, T], BF16, kind="ExternalOutput")

    F32R = mybir.dt.float32r

    with tile.TileContext(nc) as tc:
        with (
            tc.tile_pool(name="const", bufs=1) as constp,
            tc.tile_pool(name="dram", bufs=1, space="DRAM") as dramp,
            tc.tile_pool(name="qk2", bufs=2) as qk2p,
            tc.tile_pool(name="v2", bufs=2) as v2p,
            tc.tile_pool(name="wpp", bufs=1) as wpp,
            tc.tile_pool(name="w1", bufs=2) as w1p,
            tc.tile_pool(name="st1", bufs=4) as st1p,
            tc.tile_pool(name="outu", bufs=1) as outup,
            tc.tile_pool(name="pt", bufs=4) as ptp,
            tc.tile_pool(name="sum2", bufs=4) as sum2p,
            tc.tile_pool(name="dn", bufs=2) as dnp,
            tc.tile_pool(name="st3", bufs=4) as st3p,
        ):
            mask_t = constp.tile([128, 896], BF16)
            nc.sync.dma_start(mask_t[:], mask_ext[:])
            ones_t = constp.tile([128, 1], F32R)
            nc.sync.dma_start(ones_t[:], ones_ext[:].bitcast(F32R))
            bqk_t = constp.tile([128, 8], F32)
            nc.sync.dma_start(bqk_t[:], bqk_ext[:])
            bvb_t = constp.tile([128, 512], F32)
            nc.sync.dma_start(bvb_t[:], bvb_ext[:])

            qk_spill = dramp.tile([8, 128, T], BF16)
            v_spill = dramp.tile([HPC, TB, 128, 128], BF16)  # head-major
            # outU[d, 4*h + tsb, t]: normalized attention output (bf16)
            outU = outup.tile([128, 16, 512], BF16)

            # ---------------- Phase 1a: V projection ----------------
            # v in natural [t, f] layout (f = 4 heads x 128); wv/xt loads
            # interleaved so the first matmul chain starts after ~1MB of DMA.
            with tc.tile_pool(name="xt", bufs=1) as xtp:
                xt_t = xtp.tile([128, KT, T], BF16)
                with (
                    tc.tile_pool(name="wv", bufs=1) as wvp,
                    tc.tile_pool(name="psv", bufs=4, space="PSUM") as psvp,
                ):
                    wv_t = wvp.tile([128, KT, 512], BF16)
                    for kt in range(KT):
                        nc.sync.dma_start(wv_t[:, kt, :], wv_ext[kt])
                        nc.sync.dma_start(xt_t[:, kt, :], xt_ext[kt])
                    for tb in range(TB):
                        psv = psvp.tile([128, 512], F32, tag="psv")
                        for kt in range(KT):
                            nc.tensor.matmul(
                                psv[:],
                                xt_t[:, kt, 128 * tb : 128 * (tb + 1)],
                                wv_t[:, kt, :],
                                start=(kt == 0),
                                stop=(kt == KT - 1),
                            )
                        stgv = st1p.tile([128, 512], BF16, tag="stg")
                        nc.vector.tensor_add(stgv[:], psv[:], bvb_t[:])
                        for j in range(HPC):
                            nc.sync.dma_start(
                                v_spill[j, tb], stgv[:, 128 * j : 128 * (j + 1)]
                            )
                        nc.sync.dma_start(v_out_ext[tb], stgv[:])

                # ------ Phase 1b + 2 + 3 interleaved ------
                with (
                    tc.tile_pool(name="ps1", bufs=1, space="PSUM") as ps1p,
                    tc.tile_pool(name="ps_s", bufs=2, space="PSUM") as pssp,
                    tc.tile_pool(name="ps_o", bufs=1, space="PSUM") as psop,
                    tc.tile_pool(name="ps_d", bufs=1, space="PSUM") as psdp,
                ):
                    wp_t = wpp.tile([128, HPC, D], BF16)

                    def emit_qk(fb):
                        # q (fb 0-3, scaled by 1/sqrt(dh)) / k (fb 4-7) in
                        # qT/kT[f, t] layout; two tsb accumulations per pass.
                        w_t = w1p.tile([128, KT, 128], BF16, tag="w1", name=f"w_{fb}")
                        for kt in range(KT):
                            nc.sync.dma_start(w_t[:, kt, :], wqk_ext[fb, kt])
                        for half in range(2):
                            ps_a = ps1p.tile(
                                [128, 512], F32, tag="ps1_a", name=f"psa_{fb}_{half}"
                            )
                            ps_b = ps1p.tile(
                                [128, 512], F32, tag="ps1_b", name=f"psb_{fb}_{half}"
                            )
                            for kt in range(KT):
                                for tsb, ps in ((2 * half, ps_a), (2 * half + 1, ps_b)):
                                    nc.tensor.matmul(
                                        ps[:],
                                        w_t[:, kt, :],
                                        xt_t[:, kt, 512 * tsb : 512 * (tsb + 1)],
                                        start=(kt == 0),
                                        stop=(kt == KT - 1),
                                    )
                            for tsb, ps in ((2 * half, ps_a), (2 * half + 1, ps_b)):
                                stg = st1p.tile([128, 512], BF16, tag="stg")
                                nc.vector.tensor_scalar_add(
                                    stg[:], ps[:], bqk_t[:, fb : fb + 1]
                                )
                                sl = slice(512 * tsb, 512 * (tsb + 1))
                                nc.sync.dma_start(qk_spill[fb, :, sl], stg[:])
                                if fb >= 4:
                                    nc.sync.dma_start(kt_out_ext[fb - 4, :, sl], stg[:])

                    def emit_attention(h, with_proj):
                        q_t = qk2p.tile([128, T], BF16, tag="q", name=f"q_{h}")
                        nc.sync.dma_start(q_t[:], qk_spill[h])
                        k_t = qk2p.tile([128, T], BF16, tag="k", name=f"k_{h}")
                        nc.sync.dma_start(k_t[:], qk_spill[4 + h])
                        v_t = v2p.tile([128, TB, 128], BF16, tag="v", name=f"v_{h}")
                        for tb in range(TB):
                            nc.sync.dma_start(v_t[:, tb, :], v_spill[h, tb])

                        for tsb in range(TSB):
                            nt = 4 * (tsb + 1)  # causal tk blocks
                            nch = nt // 2  # chunks of 2 tk blocks
                            ps_o = psop.tile(
                                [128, 512], F32, tag="ps_o", name=f"pso_{h}_{tsb}"
                            )
                            ps_d = psdp.tile(
                                [1, 512], F32, tag="ps_d", name=f"psd_{h}_{tsb}"
                            )
                            qsl = q_t[:, 512 * tsb : 512 * (tsb + 1)]

                            # software pipeline: QK+exp of chunk c before
                            # PV/denominator of chunk c-1
                            pending = None
                            for c in range(nch + 1):
                                if c < nch:
                                    ps_c = pssp.tile(
                                        [128, 1024], F32, tag="ps_s",
                                        name=f"pss_{h}_{tsb}_{c}",
                                    )
                                    for j in range(2):
                                        tkb = 2 * c + j
                                        nc.tensor.matmul(
                                            ps_c[:, 512 * j : 512 * (j + 1)],
                                            k_t[:, 128 * tkb : 128 * (tkb + 1)],
                                            qsl,
                                            start=True,
                                            stop=True,
                                        )
                                    pt_c = ptp.tile([128, 1024], BF16, tag="pt")
                                    nc.scalar.activation(pt_c[:], ps_c[:], EXP)
                                    for j in range(2):
                                        tkb = 2 * c + j
                                        if tkb >= 4 * tsb:  # diagonal: 0/1 mask
                                            i = tkb - 4 * tsb
                                            nc.vector.tensor_mul(
                                                pt_c[:, 512 * j : 512 * (j + 1)],
                                                pt_c[:, 512 * j : 512 * (j + 1)],
                                                mask_t[
                                                    :, 384 - 128 * i : 896 - 128 * i
                                                ],
                                            )
                                    # chunk presum: one f32r denominator matmul
                                    # per chunk instead of one per block
                                    sum2 = sum2p.tile([128, 512], F32R, tag="sum2")
                                    nc.vector.tensor_add(
                                        sum2[:], pt_c[:, :512], pt_c[:, 512:]
                                    )
                                if pending is not None:
                                    pp, psum2, c0 = pending
                                    for j in range(2):
                                        tkb = 2 * c0 + j
                                        nc.tensor.matmul(
                                            ps_o[:],
                                            v_t[:, tkb, :],
                                            pp[:, 512 * j : 512 * (j + 1)],
                                            start=(tkb == 0),
                                            stop=(tkb == nt - 1),
                                        )
                                    nc.tensor.matmul(
                                        ps_d[:],
                                        ones_t[:],
                                        psum2[:],
                                        start=(c0 == 0),
                                        stop=(c0 == nch - 1),
                                    )
                                if c < nch:
                                    pending = (pt_c, sum2, c)

                            # denominator -> broadcast -> reciprocal -> normalize
                            d_sb = dnp.tile([1, 512], F32, tag="dsb")
                            nc.vector.tensor_copy(d_sb[:], ps_d[:])
                            D_t = dnp.tile([128, 512], F32, tag="Dt")
                            nc.gpsimd.partition_broadcast(D_t[:], d_sb[:])
                            r_t = dnp.tile([128, 512], F32, tag="rt")
                            nc.vector.reciprocal_approx_fast(r_t[:], D_t[:])
                            nc.vector.tensor_mul(
                                outU[:, 4 * h + tsb, :], ps_o[:], r_t[:]
                            )
                            if with_proj:
                                emit_proj(tsb)

                    def emit_proj(tsb):
                        for db in range(TB):
                            ps_y = pssp.tile(
                                [128, 512], F32, tag="ps_s", name=f"psy_{tsb}_{db}"
                            )
                            for kt in range(HPC):
                                nc.tensor.matmul(
                                    ps_y[:],
                                    wp_t[:, kt, 128 * db : 128 * (db + 1)],
                                    outU[:, 4 * kt + tsb, :],
                                    start=(kt == 0),
                                    stop=(kt == HPC - 1),
                                )
                            stg = st3p.tile([128, 512], BF16, tag="st3")
                            if db % 2 == 0:
                                nc.vector.tensor_copy(stg[:], ps_y[:])
                            else:
                                nc.scalar.activation(
                                    stg[:],
                                    ps_y[:],
                                    mybir.ActivationFunctionType.Copy,
                                )
                            nc.sync.dma_start(
                                y_out_ext[db, :, 512 * tsb : 512 * (tsb + 1)],
                                stg[:],
                            )

                    # interleave: q/k projection pairs with attention heads so
                    # attention ACT/DVE work overlaps projection matmuls and
                    # DMA stalls are absorbed.
                    emit_qk(0)
                    emit_qk(4)
                    emit_qk(1)
                    emit_qk(5)
                    for kt in range(HPC):
                        nc.sync.dma_start(wp_t[:, kt, :], wp_ext[kt])
                    emit_attention(0, False)
                    emit_qk(2)
                    emit_qk(6)
                    emit_attention(1, False)
                    emit_qk(3)
                    emit_qk(7)
                    emit_attention(2, False)
                    emit_attention(3, True)

    nc.compile()
    return nc


def _prep_in_maps(inputs, w_qkv, b_qkv, w_proj):
    scale = 1.0 / math.sqrt(DH)
    mask = np.zeros((128, 896), np.float32)
    p_idx = np.arange(128)[:, None]
    c_idx = np.arange(896)[None, :]
    mask[p_idx <= c_idx - 384] = 1.0
    ones = np.ones((128, 1), np.float32)

    wqkT = np.ascontiguousarray(w_qkv.T)  # [D, 3D]
    wpT = np.ascontiguousarray(w_proj.T)  # [D, D]

    in_maps = []
    for c in range(NCORES):
        b = c // 4
        hg = c % 4
        heads = [4 * hg + j for j in range(HPC)]

        xt = np.ascontiguousarray(inputs[b].T).reshape(KT, 128, T)

        wqk = np.empty((8, KT, 128, 128), np.float32)
        bqk = np.empty((128, 8), np.float32)
        wv = np.empty((KT, 128, HPC, 128), np.float32)
        bvb = np.empty((128, HPC, 128), np.float32)
        for j, h in enumerate(heads):
            r0 = h * 3 * DH
            wqk[j] = (wqkT[:, r0 : r0 + DH] * scale).reshape(KT, 128, 128)
            wqk[4 + j] = wqkT[:, r0 + DH : r0 + 2 * DH].reshape(KT, 128, 128)
            bqk[:, j] = b_qkv[r0 : r0 + DH] * scale
            bqk[:, 4 + j] = b_qkv[r0 + DH : r0 + 2 * DH]
            wv[:, :, j, :] = wqkT[:, r0 + 2 * DH : r0 + 3 * DH].reshape(KT, 128, 128)
            bvb[:, j, :] = b_qkv[r0 + 2 * DH : r0 + 3 * DH][None, :]
        wv = wv.reshape(KT, 128, 512)
        bvb = bvb.reshape(128, 512)

        d0 = 4 * hg * DH
        wp = np.ascontiguousarray(wpT[d0 : d0 + 512, :]).reshape(HPC, 128, D)

        in_maps.append(
            {
                "xt": xt.astype(BF16NP),
                "wqk": wqk.astype(BF16NP),
                "wv": wv.astype(BF16NP),
                "bqk": np.ascontiguousarray(bqk),
                "bvb": np.ascontiguousarray(bvb),
                "wp": wp.astype(BF16NP),
                "mask": mask.astype(BF16NP),
                "ones": ones,
            }
        )
    return in_maps


def kernel_run(inputs, w_qkv, b_qkv, w_proj, b_proj, trace=False):
    """Run the kernel; returns ((out, k, v), exec_time_ns)."""
    inputs = np.asarray(inputs, np.float32)
    w_qkv = np.asarray(w_qkv, np.float32)
    b_qkv = np.asarray(b_qkv, np.float32)
    w_proj = np.asarray(w_proj, np.float32)
    b_proj = np.asarray(b_proj, np.float32)

    if "nc" not in _CACHE:
        _CACHE["nc"] = _build()
    nc = _CACHE["nc"]

    in_maps = _prep_in_maps(inputs, w_qkv, b_qkv, w_proj)
    res = run_bass_kernel_spmd(
        nc, in_maps, core_ids=list(range(NCORES)), trace=trace
    )
    outs = res.results

    k = np.empty((B, H, T, DH), np.float32)
    v = np.empty((B, H, T, DH), np.float32)
    y = np.zeros((B, T, D), np.float32)
    for c in range(NCORES):
        b = c // 4
        hg = c % 4
        kt_o = np.asarray(outs[c]["kt_out"], dtype=np.float32)  # [4, 128, T]
        v_o = np.asarray(outs[c]["v_out"], dtype=np.float32).reshape(T, 512)
        for j in range(HPC):
            k[b, 4 * hg + j] = kt_o[j].T
            v[b, 4 * hg + j] = v_o[:, 128 * j : 128 * (j + 1)]
        y[b] += np.asarray(outs[c]["y_out"], dtype=np.float32).reshape(D, T).T
    y += b_proj[None, None, :]
    return (y, k, v), res.exec_time_ns


def kernel(inputs, w_qkv, b_qkv, w_proj, b_proj):
    out, _ = kernel_run(inputs, w_qkv, b_qkv, w_proj, b_proj, trace=False)
    return out


# revision 25
# speedup vs baseline: 1.2496x; 1.0383x over previous
"""Trainium2 Bass kernel for 16-head causal attention prefill (B=2, T=2048, D=2048).

Sharding: 8 cores = 2 batches x 4 head-groups; core c handles batch c//4,
heads 4*(c%4) .. 4*(c%4)+3 (tensor-parallel over heads within a batch).
Each core computes its heads' QKV projection, causal attention, and the
partial output projection over its 512 feature columns; the host sums the
4 partial projections per batch (the "all-reduce" of the TP scheme) and
adds the projection bias.

All matmuls run in bf16 (PE streams 2 cols/cycle; fp32 PSUM accumulation).
Causal masking is multiplicative post-exp (0/1 bf16 mask), softmax
denominators come from ones-vector matmuls, and the attention inner loop is
software-pipelined (QK of chunk c+1 issued before PV of chunk c) so the
in-order PE never stalls on the ACT exp. k/v/y ship back as bf16; the host
assembles fp32 outputs. Returns (out, k, v) matching the reference module.
"""

import math

import numpy as np
import ml_dtypes

import concourse.bass as bass  # noqa: F401
import concourse.mybir as mybir
import concourse.tile as tile
from concourse import bacc
from concourse.bass_utils import run_bass_kernel_spmd

F32 = mybir.dt.float32
BF16 = mybir.dt.bfloat16
EXP = mybir.ActivationFunctionType.Exp
BF16NP = ml_dtypes.bfloat16

B, T, D = 2, 2048, 2048
H = 16
DH = 128  # head dim
HPC = 4  # heads per core
NCORES = 8
KT = D // 128  # 16 contraction tiles
TSB = T // 512  # 4 token superblocks
TB = T // 128  # 16 token blocks

_CACHE = {}


def _build():
    nc = bacc.Bacc("TRN2", target_bir_lowering=False, debug=False, num_devices=NCORES)

    xt_ext = nc.dram_tensor("xt", [KT, 128, T], BF16, kind="ExternalInput")
    wqk_ext = nc.dram_tensor("wqk", [8, KT, 128, 128], BF16, kind="ExternalInput")
    wv_ext = nc.dram_tensor("wv", [KT, 128, 512], BF16, kind="ExternalInput")
    bqk_ext = nc.dram_tensor("bqk", [128, 8], F32, kind="ExternalInput")
    bvb_ext = nc.dram_tensor("bvb", [128, 512], F32, kind="ExternalInput")
    wp_ext = nc.dram_tensor("wp", [HPC, 128, D], BF16, kind="ExternalInput")
    mask_ext = nc.dram_tensor("mask", [128, 896], BF16, kind="ExternalInput")
    ones_ext = nc.dram_tensor("ones", [128, 128], F32, kind="ExternalInput")

    kt_out_ext = nc.dram_tensor("kt_out", [HPC, 128, T], BF16, kind="ExternalOutput")
    v_out_ext = nc.dram_tensor("v_out", [TB, 128, 512], BF16, kind="ExternalOutput")
    y_out_ext = nc.dram_tensor("y_out", [TB, 128, T], BF16, kind="ExternalOutput")

    F32R = mybir.dt.float32r

    with tile.TileContext(nc) as tc:
        with (
            tc.tile_pool(name="const", bufs=1) as constp,
            tc.tile_pool(name="dram", bufs=1, space="DRAM") as dramp,
            tc.tile_pool(name="qk2", bufs=2) as qk2p,
            tc.tile_pool(name="v2", bufs=2) as v2p,
            tc.tile_pool(name="wpp", bufs=1) as wpp,
            tc.tile_pool(name="w1", bufs=2) as w1p,
            tc.tile_pool(name="st1", bufs=4) as st1p,
            tc.tile_pool(name="outu", bufs=1) as outup,
            tc.tile_pool(name="pt", bufs=4) as ptp,
            tc.tile_pool(name="sum2", bufs=4) as sum2p,
            tc.tile_pool(name="dn", bufs=2) as dnp,
            tc.tile_pool(name="st3", bufs=4) as st3p,
        ):
            mask_t = constp.tile([128, 896], BF16)
            nc.sync.dma_start(mask_t[:], mask_ext[:])
            ones_t = constp.tile([128, 128], F32R)
            nc.sync.dma_start(ones_t[:], ones_ext[:].bitcast(F32R))
            bqk_t = constp.tile([128, 8], F32)
            nc.sync.dma_start(bqk_t[:], bqk_ext[:])
            bvb_t = constp.tile([128, 512], F32)
            nc.sync.dma_start(bvb_t[:], bvb_ext[:])

            qk_spill = dramp.tile([8, 128, T], BF16)
            v_spill = dramp.tile([HPC, TB, 128, 128], BF16)  # head-major
            # outU[d, 4*h + tsb, t]: normalized attention output (bf16)
            outU = outup.tile([128, 16, 512], BF16)

            # ---------------- Phase 1a: V projection ----------------
            # v in natural [t, f] layout (f = 4 heads x 128); wv/xt loads
            # interleaved so the first matmul chain starts after ~1MB of DMA.
            with tc.tile_pool(name="xt", bufs=1) as xtp:
                xt_t = xtp.tile([128, KT, T], BF16)
                with (
                    tc.tile_pool(name="wv", bufs=1) as wvp,
                    tc.tile_pool(name="psv", bufs=4, space="PSUM") as psvp,
                ):
                    wv_t = wvp.tile([128, KT, 512], BF16)
                    for kt in range(KT):
                        nc.sync.dma_start(wv_t[:, kt, :], wv_ext[kt])
                        nc.sync.dma_start(xt_t[:, kt, :], xt_ext[kt])
                    w_pre = {}
                    for _fb in (0, 4):
                        _w = w1p.tile(
                            [128, KT, 128], BF16, tag="w1", name=f"wpre_{_fb}"
                        )
                        for kt in range(KT):
                            nc.sync.dma_start(_w[:, kt, :], wqk_ext[_fb, kt])
                        w_pre[_fb] = _w
                    for tb in range(TB):
                        psv = psvp.tile([128, 512], F32, tag="psv")
                        for kt in range(KT):
                            nc.tensor.matmul(
                                psv[:],
                                xt_t[:, kt, 128 * tb : 128 * (tb + 1)],
                                wv_t[:, kt, :],
                                start=(kt == 0),
                                stop=(kt == KT - 1),
                            )
                        stgv = st1p.tile([128, 512], BF16, tag="stg")
                        nc.vector.tensor_add(stgv[:], psv[:], bvb_t[:])
                        for j in range(HPC):
                            nc.sync.dma_start(
                                v_spill[j, tb], stgv[:, 128 * j : 128 * (j + 1)]
                            )
                        nc.sync.dma_start(v_out_ext[tb], stgv[:])

                # ------ Phase 1b + 2 + 3 interleaved ------
                with (
                    tc.tile_pool(name="ps1", bufs=1, space="PSUM") as ps1p,
                    tc.tile_pool(name="ps_s", bufs=2, space="PSUM") as pssp,
                    tc.tile_pool(name="ps_o", bufs=1, space="PSUM") as psop,
                    tc.tile_pool(name="ps_d", bufs=1, space="PSUM") as psdp,
                ):
                    wp_t = wpp.tile([128, HPC, D], BF16)

                    def load_w(fb):
                        w_t = w1p.tile([128, KT, 128], BF16, tag="w1", name=f"w_{fb}")
                        for kt in range(KT):
                            nc.sync.dma_start(w_t[:, kt, :], wqk_ext[fb, kt])
                        return w_t

                    def emit_qk(fb):
                        # q (fb 0-3, scaled by 1/sqrt(dh)) / k (fb 4-7) in
                        # qT/kT[f, t] layout; two tsb accumulations per pass.
                        w_t = w_pre.pop(fb) if fb in w_pre else load_w(fb)
                        for half in range(2):
                            ps_a = ps1p.tile(
                                [128, 512], F32, tag="ps1_a", name=f"psa_{fb}_{half}"
                            )
                            ps_b = ps1p.tile(
                                [128, 512], F32, tag="ps1_b", name=f"psb_{fb}_{half}"
                            )
                            for kt in range(KT):
                                for tsb, ps in ((2 * half, ps_a), (2 * half + 1, ps_b)):
                                    nc.tensor.matmul(
                                        ps[:],
                                        w_t[:, kt, :],
                                        xt_t[:, kt, 512 * tsb : 512 * (tsb + 1)],
                                        start=(kt == 0),
                                        stop=(kt == KT - 1),
                                    )
                            for tsb, ps in ((2 * half, ps_a), (2 * half + 1, ps_b)):
                                stg = st1p.tile([128, 512], BF16, tag="stg")
                                nc.vector.tensor_scalar_add(
                                    stg[:], ps[:], bqk_t[:, fb : fb + 1]
                                )
                                sl = slice(512 * tsb, 512 * (tsb + 1))
                                nc.sync.dma_start(qk_spill[fb, :, sl], stg[:])
                                if fb >= 4:
                                    nc.sync.dma_start(kt_out_ext[fb - 4, :, sl], stg[:])

                    def emit_attention(h, with_proj):
                        q_t = qk2p.tile([128, T], BF16, tag="q", name=f"q_{h}")
                        nc.sync.dma_start(q_t[:], qk_spill[h])
                        k_t = qk2p.tile([128, T], BF16, tag="k", name=f"k_{h}")
                        nc.sync.dma_start(k_t[:], qk_spill[4 + h])
                        v_t = v2p.tile([128, TB, 128], BF16, tag="v", name=f"v_{h}")
                        for tb in range(TB):
                            nc.sync.dma_start(v_t[:, tb, :], v_spill[h, tb])

                        for tsb in range(TSB):
                            nt = 4 * (tsb + 1)  # causal tk blocks
                            nch = nt // 2  # chunks of 2 tk blocks
                            ps_o = psop.tile(
                                [128, 512], F32, tag="ps_o", name=f"pso_{h}_{tsb}"
                            )
                            ps_d = psdp.tile(
                                [128, 512], F32, tag="ps_d", name=f"psd_{h}_{tsb}"
                            )
                            qsl = q_t[:, 512 * tsb : 512 * (tsb + 1)]

                            # software pipeline: QK+exp of chunk c before
                            # PV/denominator of chunk c-1
                            pending = None
                            for c in range(nch + 1):
                                if c < nch:
                                    ps_c = pssp.tile(
                                        [128, 1024], F32, tag="ps_s",
                                        name=f"pss_{h}_{tsb}_{c}",
                                    )
                                    for j in range(2):
                                        tkb = 2 * c + j
                                        nc.tensor.matmul(
                                            ps_c[:, 512 * j : 512 * (j + 1)],
                                            k_t[:, 128 * tkb : 128 * (tkb + 1)],
                                            qsl,
                                            start=True,
                                            stop=True,
                                        )
                                    pt_c = ptp.tile([128, 1024], BF16, tag="pt")
                                    nc.scalar.activation(pt_c[:], ps_c[:], EXP)
                                    for j in range(2):
                                        tkb = 2 * c + j
                                        if tkb >= 4 * tsb:  # diagonal: 0/1 mask
                                            i = tkb - 4 * tsb
                                            nc.vector.tensor_mul(
                                                pt_c[:, 512 * j : 512 * (j + 1)],
                                                pt_c[:, 512 * j : 512 * (j + 1)],
                                                mask_t[
                                                    :, 384 - 128 * i : 896 - 128 * i
                                                ],
                                            )
                                    # chunk presum: one f32r denominator matmul
                                    # per chunk instead of one per block
                                    sum2 = sum2p.tile([128, 512], F32R, tag="sum2")
                                    nc.vector.tensor_add(
                                        sum2[:], pt_c[:, :512], pt_c[:, 512:]
                                    )
                                if pending is not None:
                                    pp, psum2, c0 = pending
                                    for j in range(2):
                                        tkb = 2 * c0 + j
                                        nc.tensor.matmul(
                                            ps_o[:],
                                            v_t[:, tkb, :],
                                            pp[:, 512 * j : 512 * (j + 1)],
                                            start=(tkb == 0),
                                            stop=(tkb == nt - 1),
                                        )
                                    nc.tensor.matmul(
                                        ps_d[:],
                                        ones_t[:],
                                        psum2[:],
                                        start=(c0 == 0),
                                        stop=(c0 == nch - 1),
                                    )
                                if c < nch:
                                    pending = (pt_c, sum2, c)

                            # denominator arrives already broadcast across
                            # partitions (all-ones stationary) -> reciprocal
                            r_t = dnp.tile([128, 512], F32, tag="rt")
                            nc.vector.reciprocal_approx_fast(r_t[:], ps_d[:])
                            nc.vector.tensor_mul(
                                outU[:, 4 * h + tsb, :], ps_o[:], r_t[:]
                            )
                            if with_proj:
                                emit_proj(tsb)

                    def emit_proj(tsb):
                        for db in range(TB):
                            ps_y = pssp.tile(
                                [128, 512], F32, tag="ps_s", name=f"psy_{tsb}_{db}"
                            )
                            for kt in range(HPC):
                                nc.tensor.matmul(
                                    ps_y[:],
                                    wp_t[:, kt, 128 * db : 128 * (db + 1)],
                                    outU[:, 4 * kt + tsb, :],
                                    start=(kt == 0),
                                    stop=(kt == HPC - 1),
                                )
                            stg = st3p.tile([128, 512], BF16, tag="st3")
                            if db % 2 == 0:
                                nc.vector.tensor_copy(stg[:], ps_y[:])
                            else:
                                nc.scalar.activation(
                                    stg[:],
                                    ps_y[:],
                                    mybir.ActivationFunctionType.Copy,
                                )
                            nc.sync.dma_start(
                                y_out_ext[db, :, 512 * tsb : 512 * (tsb + 1)],
                                stg[:],
                            )

                    # interleave: q/k projection pairs with attention heads so
                    # attention ACT/DVE work overlaps projection matmuls and
                    # DMA stalls are absorbed.
                    emit_qk(0)
                    emit_qk(4)
                    emit_qk(1)
                    emit_qk(5)
                    for kt in range(HPC):
                        nc.sync.dma_start(wp_t[:, kt, :], wp_ext[kt])
                    emit_attention(0, False)
                    emit_qk(2)
                    emit_qk(6)
                    emit_attention(1, False)
                    emit_qk(3)
                    emit_qk(7)
                    emit_attention(2, False)
                    emit_attention(3, True)

    nc.compile()
    return nc


def _prep_in_maps(inputs, w_qkv, b_qkv, w_proj):
    scale = 1.0 / math.sqrt(DH)
    mask = np.zeros((128, 896), np.float32)
    p_idx = np.arange(128)[:, None]
    c_idx = np.arange(896)[None, :]
    mask[p_idx <= c_idx - 384] = 1.0
    ones = np.ones((128, 128), np.float32)

    wqkT = np.ascontiguousarray(w_qkv.T)  # [D, 3D]
    wpT = np.ascontiguousarray(w_proj.T)  # [D, D]

    in_maps = []
    for c in range(NCORES):
        b = c // 4
        hg = c % 4
        heads = [4 * hg + j for j in range(HPC)]

        xt = np.ascontiguousarray(inputs[b].T).reshape(KT, 128, T)

        wqk = np.empty((8, KT, 128, 128), np.float32)
        bqk = np.empty((128, 8), np.float32)
        wv = np.empty((KT, 128, HPC, 128), np.float32)
        bvb = np.empty((128, HPC, 128), np.float32)
        for j, h in enumerate(heads):
            r0 = h * 3 * DH
            wqk[j] = (wqkT[:, r0 : r0 + DH] * scale).reshape(KT, 128, 128)
            wqk[4 + j] = wqkT[:, r0 + DH : r0 + 2 * DH].reshape(KT, 128, 128)
            bqk[:, j] = b_qkv[r0 : r0 + DH] * scale
            bqk[:, 4 + j] = b_qkv[r0 + DH : r0 + 2 * DH]
            wv[:, :, j, :] = wqkT[:, r0 + 2 * DH : r0 + 3 * DH].reshape(KT, 128, 128)
            bvb[:, j, :] = b_qkv[r0 + 2 * DH : r0 + 3 * DH][None, :]
        wv = wv.reshape(KT, 128, 512)
        bvb = bvb.reshape(128, 512)

        d0 = 4 * hg * DH
        wp = np.ascontiguousarray(wpT[d0 : d0 + 512, :]).reshape(HPC, 128, D)

        in_maps.append(
            {
                "xt": xt.astype(BF16NP),
                "wqk": wqk.astype(BF16NP),
                "wv": wv.astype(BF16NP),
                "bqk": np.ascontiguousarray(bqk),
                "bvb": np.ascontiguousarray(bvb),
                "wp": wp.astype(BF16NP),
                "mask": mask.astype(BF16NP),
                "ones": ones,
            }
        )
    return in_maps


def kernel_run(inputs, w_qkv, b_qkv, w_proj, b_proj, trace=False):
    """Run the kernel; returns ((out, k, v), exec_time_ns)."""
    inputs = np.asarray(inputs, np.float32)
    w_qkv = np.asarray(w_qkv, np.float32)
    b_qkv = np.asarray(b_qkv, np.float32)
    w_proj = np.asarray(w_proj, np.float32)
    b_proj = np.asarray(b_proj, np.float32)

    if "nc" not in _CACHE:
        _CACHE["nc"] = _build()
    nc = _CACHE["nc"]

    in_maps = _prep_in_maps(inputs, w_qkv, b_qkv, w_proj)
    res = run_bass_kernel_spmd(
        nc, in_maps, core_ids=list(range(NCORES)), trace=trace
    )
    outs = res.results

    k = np.empty((B, H, T, DH), np.float32)
    v = np.empty((B, H, T, DH), np.float32)
    y = np.zeros((B, T, D), np.float32)
    for c in range(NCORES):
        b = c // 4
        hg = c % 4
        kt_o = np.asarray(outs[c]["kt_out"], dtype=np.float32)  # [4, 128, T]
        v_o = np.asarray(outs[c]["v_out"], dtype=np.float32).reshape(T, 512)
        for j in range(HPC):
            k[b, 4 * hg + j] = kt_o[j].T
            v[b, 4 * hg + j] = v_o[:, 128 * j : 128 * (j + 1)]
        y[b] += np.asarray(outs[c]["y_out"], dtype=np.float32).reshape(D, T).T
    y += b_proj[None, None, :]
    return (y, k, v), res.exec_time_ns


def kernel(inputs, w_qkv, b_qkv, w_proj, b_proj):
    out, _ = kernel_run(inputs, w_qkv, b_qkv, w_proj, b_proj, trace=False)
    return out
